# revision 23
# baseline (speedup 1.0000x reference)
"""Trainium2 Bass kernel for nn_DirectMaskedProjection (masked projection).

kernel(**inputs): FULL inputs -> FULL [1,128,128] image. 8 NeuronCores,
data-parallel over 16-row h-blocks of the output image.

Host packs the binary mask volume to 1 bit/voxel (BLAS matvec with bit
weights) and shards it across cores (32KB/core over the wire); all
small params travel as one [1,352] f32 row per core. On device: an
AllGather reassembles the full packed volume, bits are unpacked and
the 2x2 (y,x) corner bits of mask_vol packed into a base-4 code (bf16,
[128 z, 16384]); per d-plane ap_gather each point's cell-code
z-column; one-hot-select slices z0/z0+1 into PSUM rows via bf16
matmuls (z0 row replicated across partitions with a PE row-selector
matmul); arithmetically unpack corner bits and apply the exact
trilinear!=0 OR logic; evaluate the field MLP; reduce over depth with
a ones-matmul.

The jax.jit shard_map wrapper around the compiled module is built once
and cached: re-tracing it per call (as run_bass_kernel_spmd does) costs
~450ms/call. Per call the host does ~1ms of packing, ships ~330KB, and
pays one axon round trip (~55-120ms depending on tunnel conditions).
"""
import numpy as np

H, W, D = 128, 128, 64
HB = 16
N_CORES = 8
HIDDEN = 64
NP = HB * W          # 2048 points per d-plane per core
NH = NP // 2         # tail half-pass width

# packed param row offsets
OT, OW1, OB1, OW2, OB2, OH0, NPAR = 0, 16, 208, 272, 336, 337, 352

_BITW = (1 << np.arange(8)).astype(np.float32)

_CACHE = {}


def _build():
    import concourse.mybir as mybir
    import concourse.tile as tile
    from concourse import bacc
    import contextlib

    dt = mybir.dt
    f32, i32, i16, bf16 = dt.float32, dt.int32, dt.int16, dt.bfloat16
    u8 = dt.uint8
    Alu = mybir.AluOpType
    Act = mybir.ActivationFunctionType

    nc = bacc.Bacc("TRN2", target_bir_lowering=False, debug=False,
                   num_devices=N_CORES)
    volp = nc.declare_dram_parameter("volp", [128 // N_CORES, 2048], u8,
                                     isOutput=False)
    pard = nc.declare_dram_parameter("par", [1, NPAR], f32, isOutput=False)
    imgd = nc.declare_dram_parameter("img", [1, NP], f32, isOutput=True)

    with tile.TileContext(nc) as tc, contextlib.ExitStack() as ctx:
        vp = ctx.enter_context(tc.tile_pool(name="vp", bufs=1))
        per = ctx.enter_context(tc.tile_pool(name="per", bufs=1))
        wk = ctx.enter_context(tc.tile_pool(name="wk", bufs=1))
        tl = ctx.enter_context(tc.tile_pool(name="tl", bufs=1))
        psp = ctx.enter_context(tc.tile_pool(name="psp", bufs=1, space="PSUM"))
        dram = ctx.enter_context(tc.tile_pool(name="dram", bufs=1,
                                              space="DRAM"))

        # ---- small inputs: one row, DMA-broadcast across partitions ----
        pall = per.tile([128, NPAR], f32)
        nc.gpsimd.dma_start(out=pall[:], in_=pard[:].to_broadcast([128, NPAR]))
        tmt = pall

        def T(r, c, p):
            k = OT + 4 * r + c
            return pall[0:p, k:k + 1]

        # ---- phase 1: all-gather volume shard, load + unpack + pack ----
        vin = dram.tile([128 // N_CORES, 2048], u8)
        vout = dram.tile([128, 2048], u8)
        nc.gpsimd.dma_start(out=vin[:], in_=volp[:])
        nc.gpsimd.collective_compute(
            "AllGather", Alu.bypass,
            replica_groups=[list(range(N_CORES))],
            ins=[vin.opt()], outs=[vout.opt()])
        vu8 = wk.tile([128, 2048], u8, tag="vu8")
        nc.gpsimd.dma_start(out=vu8[:], in_=vout[:])
        vbf = vp.tile([128, 16384], bf16)
        vbv = vbf[:].rearrange("p (j k) -> p j k", k=8)
        for cc in range(2):
            cs = slice(1024 * cc, 1024 * (cc + 1))
            vi = wk.tile([128, 1024], i16, tag="vi")
            nc.vector.tensor_copy(out=vi[:], in_=vu8[:, cs])
            bi = wk.tile([128, 1024], i16, tag="vbi")
            for kk in range(8):
                nc.vector.tensor_scalar(out=bi[:], in0=vi[:], scalar1=kk,
                                        scalar2=1,
                                        op0=Alu.logical_shift_right,
                                        op1=Alu.bitwise_and)
                nc.vector.tensor_copy(out=vbv[:, cs, kk:kk + 1],
                                      in_=bi[:].unsqueeze(2))
        At = vp.tile([128, 16384], bf16)
        nc.vector.scalar_tensor_tensor(
            out=At[:, 0:16383], in0=vbf[:, 1:16384], scalar=4.0,
            in1=vbf[:, 0:16383], op0=Alu.mult, op1=Alu.add)
        av = At[:].rearrange("p (y x) -> p y x", x=128)[:, :, 127:128]
        vv = vbf[:].rearrange("p (y x) -> p y x", x=128)[:, :, 127:128]
        nc.vector.tensor_copy(out=av, in_=vv)                # col x=127 fix
        Bt = vbf                                             # reuse storage
        nc.vector.scalar_tensor_tensor(
            out=Bt[:, 0:16256], in0=At[:, 128:16384], scalar=16.0,
            in1=At[:, 0:16256], op0=Alu.mult, op1=Alu.add)
        nc.vector.tensor_copy(out=Bt[:, 16256:16384], in_=At[:, 16256:16384])

        # ---- static columns ----
        pci = per.tile([128, 1], i32)
        nc.gpsimd.iota(pci[:], pattern=[[0, 1]], channel_multiplier=1)
        pcf = per.tile([128, 1], f32)
        nc.vector.tensor_copy(out=pcf[:], in_=pci[:])
        t16 = per.tile([128, 1], f32)
        t16i = per.tile([128, 1], i32)
        nc.vector.tensor_scalar_mul(t16[:], pcf[:], 1.0 / 16.0)
        nc.vector.tensor_copy(out=t16i[:], in_=t16[:])
        tfc = per.tile([128, 1], f32)
        nc.vector.tensor_copy(out=tfc[:], in_=t16i[:])
        ltc = per.tile([128, 1], f32)
        nc.vector.tensor_tensor(out=ltc[:], in0=t16[:], in1=tfc[:],
                                op=Alu.is_lt)
        nc.vector.tensor_sub(tfc[:], tfc[:], ltc[:])
        hcol = per.tile([128, 1], f32)                       # p % 16
        nc.vector.scalar_tensor_tensor(out=hcol[:], in0=tfc[:], scalar=-16.0,
                                       in1=pcf[:], op0=Alu.mult, op1=Alu.add)
        h0m = per.tile([128, 1], f32)
        nc.vector.tensor_scalar_add(h0m[:], pall[:, OH0:OH0 + 1], -63.5)
        pxcol = per.tile([128, 1], f32)                      # px(h(p))
        nc.vector.tensor_add(pxcol[:], hcol[:], h0m[:])
        wri2 = per.tile([128, W], i32)
        nc.gpsimd.iota(wri2[:], pattern=[[1, W]], channel_multiplier=0)
        pyrow = per.tile([128, W], f32)                      # py(w) = w-63.5
        nc.vector.tensor_copy(out=pyrow[:], in_=wri2[:])
        nc.vector.tensor_scalar_add(pyrow[:], pyrow[:], -63.5)
        negp = per.tile([128, 1], f32)
        onemp = per.tile([128, 1], f32)
        nc.vector.tensor_scalar_mul(negp[:], pcf[:], -1.0)
        nc.vector.tensor_scalar(out=onemp[:], in0=pcf[:], scalar1=-1.0,
                                scalar2=1.0, op0=Alu.mult, op1=Alu.add)
        bigZ = per.tile([128, 255], bf16)
        nc.vector.memset(bigZ[:], 0.0)
        nc.vector.memset(bigZ[:, 127:128], 1.0)
        idr = per.tile([64, 64], i32)
        nc.gpsimd.iota(idr[:], pattern=[[0, 64]], channel_multiplier=1)
        idc = per.tile([64, 64], i32)
        nc.gpsimd.iota(idc[:], pattern=[[1, 64]], channel_multiplier=0)
        idrf = per.tile([64, 64], f32)
        nc.vector.tensor_copy(out=idrf[:], in_=idr[:])
        idcf = per.tile([64, 64], f32)
        nc.vector.tensor_copy(out=idcf[:], in_=idc[:])
        id64 = per.tile([64, 64], bf16)
        nc.vector.tensor_tensor(out=id64[:], in0=idrf[:], in1=idcf[:],
                                op=Alu.is_equal)
        ones64 = per.tile([64, 1], f32)
        nc.vector.memset(ones64[:], 1.0)
        dci = per.tile([64, 1], i32)
        nc.gpsimd.iota(dci[:], pattern=[[0, 1]], channel_multiplier=1)
        pzc = per.tile([64, 1], f32)
        nc.vector.tensor_copy(out=pzc[:], in_=dci[:])
        nc.vector.tensor_scalar(out=pzc[:], in0=pzc[:], scalar1=2.0,
                                scalar2=-63.0, op0=Alu.mult, op1=Alu.add)

        z0t = per.tile([64, NP], f32)
        z0b = per.tile([64, NP], bf16)
        P2b = per.tile([64, NP], bf16)
        Scp = per.tile([128, NP], f32)
        Scp1 = per.tile([64, NP], f32)
        imgrow = per.tile([1, NP], f32)

        def S(name):
            return tl.tile([64, NH], f32, tag=name, name=name)

        def Si(name):
            return tl.tile([64, NH], i32, tag=name, name=name + "_i")

        def floor_to(src_ap, out_ap, itag, ltag):
            ti = Si(itag)
            nc.vector.tensor_copy(out=ti[:], in_=src_ap)
            nc.vector.tensor_copy(out=out_ap, in_=ti[:])
            ltm = S(ltag)
            nc.vector.tensor_tensor(out=ltm[:], in0=src_ap, in1=out_ap,
                                    op=Alu.is_lt)
            nc.vector.tensor_tensor(out=out_ap, in0=out_ap, in1=ltm[:],
                                    op=Alu.subtract)

        def ramps(hh):
            ri = Si("ti")
            hrf, wrf = S("hrf"), S("wrf")
            nc.gpsimd.iota(ri[:], pattern=[[0, W // 2], [1, HB]],
                           channel_multiplier=0)
            nc.vector.tensor_copy(out=hrf[:], in_=ri[:])
            nc.gpsimd.iota(ri[:], pattern=[[1, W // 2], [0, HB]],
                           base=(W // 2) * hh, channel_multiplier=0)
            nc.vector.tensor_copy(out=wrf[:], in_=ri[:])
            nc.vector.tensor_scalar(out=hrf[:], in0=hrf[:], scalar1=h0m[0:64, 0:1],
                                    scalar2=0.0, op0=Alu.add, op1=Alu.add)
            nc.vector.tensor_scalar_add(wrf[:], wrf[:], -63.5)
            return hrf, wrf

        def qcoord(c, hrf, wrf, dst):
            nc.vector.tensor_scalar(out=dst[:], in0=wrf[:], scalar1=T(c, 1, 64),
                                    scalar2=0.0, op0=Alu.mult, op1=Alu.add)
            nc.vector.scalar_tensor_tensor(
                out=dst[:], in0=hrf[:], scalar=T(c, 0, 64), in1=dst[:],
                op0=Alu.mult, op1=Alu.add)
            nc.vector.scalar_tensor_tensor(
                out=dst[:], in0=pzc[:].to_broadcast([64, NH]),
                scalar=T(c, 2, 64), in1=dst[:], op0=Alu.mult, op1=Alu.add)
            nc.vector.tensor_scalar(out=dst[:], in0=dst[:], scalar1=T(c, 3, 64),
                                    scalar2=0.0, op0=Alu.add, op1=Alu.add)

        def vox(src_ap, dst_ap):
            # ((q/63.5)*0.5 + 0.5)*127 == q + 63.5 exactly (127/127 = 1)
            nc.vector.tensor_scalar(out=dst_ap, in0=src_ap, scalar1=63.5,
                                    scalar2=None, op0=Alu.add)
            nc.vector.tensor_scalar_max(dst_ap, dst_ap, -1.5)
            nc.vector.tensor_scalar_min(dst_ap, dst_ap, 129.5)

        # ---- z0 batch tile, built in halves ----
        for hh in range(2):
            fs = slice(NH * hh, NH * (hh + 1))
            hrf, wrf = ramps(hh)
            u = S("u")
            qcoord(2, hrf, wrf, u)
            cl = S("cl")
            vox(u[:], cl[:])
            floor_to(cl[:], z0t[:, fs], "ti", "lt")
        nc.vector.tensor_copy(out=z0b[:], in_=z0t[:])
        for hh in range(2):
            fs = slice(NH * hh, NH * (hh + 1))
            hrf, wrf = ramps(hh)
            u = S("u")
            qcoord(0, hrf, wrf, u)
            cl = S("cl")
            vox(u[:], cl[:])
            c0 = S("hi")
            floor_to(cl[:], c0[:], "ti", "lt")
            nc.vector.tensor_scalar_max(c0[:], c0[:], 0.0)
            nc.vector.tensor_scalar_min(c0[:], c0[:], 127.0)
            u2 = S("u")
            nc.vector.tensor_scalar_mul(u2[:], c0[:], 0.5)
            hf = S("cl")
            floor_to(u2[:], hf[:], "ti", "lt")
            nc.vector.scalar_tensor_tensor(out=P2b[0:64, fs], in0=hf[:],
                                           scalar=-2.0, in1=c0[:],
                                           op0=Alu.mult, op1=Alu.add)

        # ---- phase 2: per-plane gather + z-select into PSUM ----
        t23 = per.tile([128, 2], f32)            # T(c,3) + 63.5, c in {0,1}
        for c in (0, 1):
            nc.vector.tensor_scalar(out=t23[:, c:c + 1],
                                    in0=tmt[:, 4 * c + 3:4 * c + 4],
                                    scalar1=63.5, scalar2=None, op0=Alu.add)
        psS = psp.tile([128, NP], f32)
        zrep = psp.tile([128, NH], f32)
        for dcp in range(D):
            pzv = 2.0 * dcp - 63.0
            flrs = []
            for c in (0, 1):
                u = wk.tile([128, W], f32, tag="pl_u")
                nc.vector.tensor_scalar(out=u[:], in0=pyrow[:], scalar1=T(c, 1, 128),
                                        scalar2=0.0, op0=Alu.mult, op1=Alu.add)
                nc.vector.scalar_tensor_tensor(
                    out=u[:], in0=pxcol[:].to_broadcast([128, W]),
                    scalar=T(c, 0, 128), in1=u[:], op0=Alu.mult, op1=Alu.add)
                # z term folded with T(c,3)+63.5 (voxel = raw + 63.5, see vox)
                szc = wk.tile([128, 1], f32, tag="pl_s")
                nc.vector.tensor_scalar(
                    out=szc[:], in0=tmt[:, 4 * c + 2:4 * c + 3],
                    scalar1=pzv, scalar2=t23[:, c:c + 1],
                    op0=Alu.mult, op1=Alu.add)
                nc.vector.tensor_scalar(out=u[:], in0=u[:], scalar1=szc[:],
                                        scalar2=0.0, op0=Alu.add, op1=Alu.add)
                nc.vector.tensor_scalar_max(u[:], u[:], -1.5)
                nc.vector.tensor_scalar_min(u[:], u[:], 129.5)
                ti = wk.tile([128, W], i32, tag="pl_i")
                nc.vector.tensor_copy(out=ti[:], in_=u[:])
                fl = wk.tile([128, W], f32, tag=f"pl_f{c}")
                nc.vector.tensor_copy(out=fl[:], in_=ti[:])
                ltm = wk.tile([128, W], f32, tag="pl_l")
                nc.vector.tensor_tensor(out=ltm[:], in0=u[:], in1=fl[:],
                                        op=Alu.is_lt)
                nc.vector.tensor_sub(fl[:], fl[:], ltm[:])
                nc.vector.tensor_scalar_max(fl[:], fl[:], 0.0)
                nc.vector.tensor_scalar_min(fl[:], fl[:], 127.0)
                flrs.append(fl)
            cellv = wk.tile([128, W], f32, tag="pl_c")
            nc.vector.scalar_tensor_tensor(out=cellv[:], in0=flrs[1][:],
                                           scalar=128.0, in1=flrs[0][:],
                                           op0=Alu.mult, op1=Alu.add)
            half = wk.tile([128, W], f32, tag="pl_u")
            nc.vector.tensor_scalar_mul(half[:], cellv[:], 0.5)
            hfi = wk.tile([128, W], i32, tag="pl_i")
            nc.vector.tensor_copy(out=hfi[:], in_=half[:])
            hff = wk.tile([128, W], f32, tag="pl_hf")
            nc.vector.tensor_copy(out=hff[:], in_=hfi[:])
            hlt = wk.tile([128, W], f32, tag="pl_l")
            nc.vector.tensor_tensor(out=hlt[:], in0=half[:], in1=hff[:],
                                    op=Alu.is_lt)
            nc.vector.tensor_sub(hff[:], hff[:], hlt[:])
            idx16 = wk.tile([128, W], i16, tag="pl_x")
            nc.vector.tensor_copy(out=idx16[:], in_=hff[:])

            C = wk.tile([128, 2 * NP], bf16, tag="C")
            nc.gpsimd.ap_gather(C[:], Bt[:], idx16[:], channels=128,
                                num_elems=8192, d=2, num_idxs=NP)
            Cv = C[:].rearrange("p (i d) -> p i d", d=2)

            t0 = wk.tile([128, NH], f32, tag="t0")
            E0 = wk.tile([128, NP], bf16, tag="E0")
            E1 = wk.tile([128, NP], bf16, tag="E1")
            Csel = wk.tile([128, NP], bf16, tag="Csel")
            parh = wk.tile([128, NH], bf16, tag="parh")
            sel = wk.tile([64, 128], bf16, tag="sel")
            nc.vector.tensor_copy(
                out=sel[:, :],
                in_=id64[0:64, dcp:dcp + 1].to_broadcast([64, 128]))
            for hz in range(2):
                zfs = slice(NH * hz, NH * (hz + 1))
                for qq in range(2):
                    qs_ = slice(512 * qq, 512 * (qq + 1))
                    nc.tensor.matmul(zrep[:, qs_], sel[:, :],
                                     z0b[:, NH * hz + 512 * qq:
                                         NH * hz + 512 * (qq + 1)],
                                     start=True, stop=True)
                nc.scalar.activation(out=t0[:, :], in_=zrep[:, :],
                                     func=Act.Abs, bias=negp[:], scale=1.0)
                nc.scalar.activation(out=E0[:, zfs], in_=t0[:, :],
                                     func=Act.Relu, bias=1.0, scale=-1.0)
                nc.scalar.activation(out=t0[:, :], in_=zrep[:, :],
                                     func=Act.Abs, bias=onemp[:], scale=1.0)
                nc.scalar.activation(out=E1[:, zfs], in_=t0[:, :],
                                     func=Act.Relu, bias=1.0, scale=-1.0)
                for qq in range(2):
                    qs_ = slice(512 * qq, 512 * (qq + 1))
                    nc.tensor.matmul(zrep[:, qs_], sel[:, :],
                                     P2b[:, NH * hz + 512 * qq:
                                         NH * hz + 512 * (qq + 1)],
                                     start=True, stop=True)
                nc.vector.tensor_copy(out=parh[:, :], in_=zrep[:, :])
                d01 = Cv[:, zfs, 0:1]
                d11 = Cv[:, zfs, 1:2]
                csv = Csel[:, zfs].unsqueeze(2)
                dif = wk.tile([128, NH], bf16, tag="dif")
                difv = dif[:].unsqueeze(2)
                nc.vector.tensor_tensor(out=difv, in0=d11, in1=d01,
                                        op=Alu.subtract)
                nc.vector.tensor_tensor(out=difv, in0=difv,
                                        in1=parh[:, :].unsqueeze(2),
                                        op=Alu.mult)
                nc.vector.tensor_tensor(out=csv, in0=difv, in1=d01,
                                        op=Alu.add)
            M0 = wk.tile([128, NP], bf16, tag="M0")
            nc.vector.tensor_mul(M0[:], Csel[:], E0[:])
            M1 = wk.tile([128, NP], bf16, tag="M1")
            nc.vector.tensor_mul(M1[:], Csel[:], E1[:])
            for si, M in ((0, M0), (1, M1)):
                j = dcp + 64 * si
                lhs = bigZ[:, 127 - j:255 - j]
                for ch in range(4):
                    cs = slice(512 * ch, 512 * (ch + 1))
                    nc.tensor.matmul(psS[:, cs], lhs, M[:, cs],
                                     start=(dcp == 0 and si == 0),
                                     stop=(dcp == D - 1 and si == 1))

        nc.vector.tensor_copy(out=Scp[:], in_=psS[:])
        nc.gpsimd.dma_start(out=Scp1[:], in_=Scp[64:128, :])

        # ---- phase 3: tail, two half-passes ----
        psI = psp.tile([1, NH], f32)
        for hh in range(2):
            fs = slice(NH * hh, NH * (hh + 1))
            hrf, wrf = ramps(hh)
            u = S("u")
            cl = S("cl")
            qcoord(2, hrf, wrf, u)
            vox(u[:], cl[:])
            c0 = S("hi")
            floor_to(cl[:], c0[:], "ti", "lt")
            gz = S("gz")
            nc.vector.tensor_sub(cl[:], cl[:], c0[:])
            nc.vector.tensor_scalar(out=gz[:], in0=cl[:], scalar1=0.0,
                                    scalar2=None, op0=Alu.is_gt)
            ab = {}
            for c, nm in ((1, "y"), (0, "x")):
                qcoord(c, hrf, wrf, u)
                vox(u[:], cl[:])
                floor_to(cl[:], c0[:], "ti", "lt")
                g = S("g")
                nc.vector.tensor_sub(cl[:], cl[:], c0[:])
                nc.vector.tensor_scalar(out=g[:], in0=cl[:], scalar1=0.0,
                                        scalar2=None, op0=Alu.is_gt)
                ei = S("lt")
                nc.vector.tensor_scalar(out=ei[:], in0=c0[:], scalar1=0.0,
                                        scalar2=None, op0=Alu.is_ge)
                nc.vector.tensor_scalar(out=cl[:], in0=c0[:], scalar1=127.0,
                                        scalar2=None, op0=Alu.is_le)
                nc.vector.tensor_mul(ei[:], ei[:], cl[:])
                nc.vector.tensor_scalar(out=cl[:], in0=c0[:], scalar1=-1.0,
                                        scalar2=None, op0=Alu.is_equal)
                al = S("al" + nm)
                nc.vector.tensor_mul(al[:], cl[:], g[:])
                nc.vector.tensor_add(al[:], al[:], ei[:])
                be = S("be" + nm)
                nc.vector.tensor_mul(be[:], ei[:], g[:])
                ab[nm] = (al, be)

            def unpack(Sap, xv_tag):
                t = S("u")
                nc.vector.tensor_scalar_mul(t[:], Sap, 1.0 / 16.0)
                hi = S("hi")
                floor_to(t[:], hi[:], "ti", "lt")
                lo = S("cl")
                nc.vector.scalar_tensor_tensor(out=lo[:], in0=hi[:],
                                               scalar=-16.0, in1=Sap,
                                               op0=Alu.mult, op1=Alu.add)
                yt = S("g")
                nc.vector.tensor_mul(yt[:], ab["y"][1][:], hi[:])
                nc.vector.tensor_mul(lo[:], ab["y"][0][:], lo[:])
                nc.vector.tensor_add(yt[:], yt[:], lo[:])
                nc.vector.tensor_scalar_mul(t[:], yt[:], 0.25)
                floor_to(t[:], hi[:], "ti", "lt")
                nc.vector.scalar_tensor_tensor(out=lo[:], in0=hi[:],
                                               scalar=-4.0, in1=yt[:],
                                               op0=Alu.mult, op1=Alu.add)
                xv = S(xv_tag)
                nc.vector.tensor_mul(xv[:], ab["x"][1][:], hi[:])
                nc.vector.tensor_mul(lo[:], ab["x"][0][:], lo[:])
                nc.vector.tensor_add(xv[:], xv[:], lo[:])
                return xv

            xv0 = unpack(Scp[0:64, fs], "wrf")
            xv1 = unpack(Scp1[0:64, fs], "u")
            mask = S("cl")
            nc.vector.tensor_mul(mask[:], gz[:], xv1[:])
            nc.vector.tensor_add(mask[:], mask[:], xv0[:])
            nc.vector.tensor_scalar(out=mask[:], in0=mask[:], scalar1=0.0,
                                    scalar2=None, op0=Alu.is_gt)

            hrf, wrf = ramps(hh)
            q0, q1, q2, q3 = S("alx"), S("bex"), S("aly"), S("bey")
            for c, dst in ((0, q0), (1, q1), (2, q2), (3, q3)):
                qcoord(c, hrf, wrf, dst)
            rw = S("hi")
            nc.vector.reciprocal(rw[:], q3[:])
            for qq in (q0, q1, q2):
                nc.vector.tensor_mul(qq[:], qq[:], rw[:])
            pot = S("gz")
            nc.vector.memset(pot[:], 0.0)
            hu = S("u")
            for uu in range(HIDDEN):
                nc.scalar.activation(
                    out=hu[:], in_=q0[:], func=Act.Identity,
                    bias=pall[0:64, OB1 + uu:OB1 + uu + 1],
                    scale=pall[0:64, OW1 + uu:OW1 + uu + 1])
                nc.vector.scalar_tensor_tensor(
                    out=hu[:], in0=q1[:],
                    scalar=pall[0:64, OW1 + HIDDEN + uu:OW1 + HIDDEN + uu + 1],
                    in1=hu[:], op0=Alu.mult, op1=Alu.add)
                nc.vector.scalar_tensor_tensor(
                    out=hu[:], in0=q2[:],
                    scalar=pall[0:64, OW1 + 2 * HIDDEN + uu:
                                OW1 + 2 * HIDDEN + uu + 1],
                    in1=hu[:], op0=Alu.mult, op1=Alu.add)
                nc.scalar.activation(out=hu[:], in_=hu[:], func=Act.Relu)
                nc.vector.scalar_tensor_tensor(
                    out=pot[:], in0=hu[:],
                    scalar=pall[0:64, OW2 + uu:OW2 + uu + 1],
                    in1=pot[:], op0=Alu.mult, op1=Alu.add)
            nc.vector.tensor_scalar(out=pot[:], in0=pot[:],
                                    scalar1=pall[0:64, OB2:OB2 + 1],
                                    scalar2=0.0, op0=Alu.add, op1=Alu.add)
            nc.vector.tensor_mul(pot[:], pot[:], mask[:])
            for ch in range(2):
                cs = slice(512 * ch, 512 * (ch + 1))
                nc.tensor.matmul(psI[:, cs], ones64[:], pot[:, cs],
                                 start=True, stop=True)
            nc.scalar.activation(out=imgrow[:, fs], in_=psI[:],
                                 func=Act.Copy, scale=2.0)

        nc.gpsimd.dma_start(out=imgd[:], in_=imgrow[:])

    nc.compile()
    return nc


def _make_runner(nc):
    """Wrap the compiled Bass module in a cached jax.jit shard_map callable
    (the same lowering run_bass_kernel_spmd uses under axon, hoisted out of
    the per-call path so the shard_map is traced exactly once)."""
    import jax
    import concourse.mybir as mybir
    from concourse.bass2jax import (_bass_exec_p, install_neuronx_cc_hook,
                                    partition_id_tensor)
    from jax.sharding import Mesh, PartitionSpec
    from jax.experimental.shard_map import shard_map

    install_neuronx_cc_hook()
    partition_name = (nc.partition_id_tensor.name
                      if nc.partition_id_tensor else None)
    in_names, out_names, out_avals, zero_outs = [], [], [], []
    for alloc in nc.m.functions[0].allocations:
        if not isinstance(alloc, mybir.MemoryLocationSet):
            continue
        name = alloc.memorylocations[0].name
        if alloc.kind == "ExternalInput":
            if name != partition_name:
                in_names.append(name)
        elif alloc.kind == "ExternalOutput":
            shape = tuple(alloc.tensor_shape)
            dtype = mybir.dt.np(alloc.dtype)
            out_names.append(name)
            out_avals.append(jax.core.ShapedArray(shape, dtype))
            zero_outs.append(np.zeros(shape, dtype))
    n_params = len(in_names)
    n_outs = len(out_avals)
    in_names_full = in_names + out_names + (
        [partition_name] if partition_name else [])
    donate = tuple(range(n_params, n_params + n_outs))

    def _body(*args):
        operands = list(args)
        if partition_name is not None:
            operands.append(partition_id_tensor())
        return tuple(_bass_exec_p.bind(
            *operands, out_avals=tuple(out_avals),
            in_names=tuple(in_names_full), out_names=tuple(out_names),
            lowering_input_output_aliases=(), sim_require_finite=True,
            sim_require_nnan=True, nc=nc))

    devices = jax.devices()[:N_CORES]
    mesh = Mesh(np.asarray(devices), ("core",))
    sharded = jax.jit(
        shard_map(_body, mesh=mesh,
                  in_specs=(PartitionSpec("core"),) * (n_params + n_outs),
                  out_specs=(PartitionSpec("core"),) * n_outs,
                  check_rep=False),
        donate_argnums=donate, keep_unused=True)

    def run(per_input):
        # per_input: name -> [N_CORES*rows, cols] stacked per-core arrays
        cin = [per_input[n] for n in in_names]
        czeros = [np.zeros((N_CORES * z.shape[0], *z.shape[1:]), z.dtype)
                  for z in zero_outs]
        outs = sharded(*cin, *czeros)
        return {name: np.asarray(o) for name, o in zip(out_names, outs)}

    return run


def kernel(**inputs):
    if "nc" not in _CACHE:
        _CACHE["nc"] = _build()
    if "run" not in _CACHE:
        try:
            _CACHE["run"] = _make_runner(_CACHE["nc"])
        except Exception:
            _CACHE["run"] = None   # fall back to run_bass_kernel_spmd
    run = _CACHE["run"]

    # mask_vol is binary 0.0/1.0 (the device-side base-4 code arithmetic
    # already relies on this); a BLAS matvec with bit weights packs it
    # ~7x faster than np.packbits
    m = np.asarray(inputs["mask_vol"], np.float32)
    volp = (m.reshape(128, 2048, 8) @ _BITW).astype(np.uint8)
    par = np.zeros((N_CORES, NPAR), np.float32)
    par[:, OT:OT + 16] = np.asarray(
        inputs["transform_matrix"], np.float32).reshape(16)
    par[:, OW1:OW1 + 3 * HIDDEN] = np.asarray(
        inputs["W1"], np.float32).reshape(3 * HIDDEN)
    par[:, OB1:OB1 + HIDDEN] = np.asarray(inputs["b1"], np.float32)
    par[:, OW2:OW2 + HIDDEN] = np.asarray(
        inputs["W2"], np.float32).reshape(HIDDEN)
    par[:, OB2] = np.float32(np.asarray(inputs["b2"]).reshape(())[()])
    par[:, OH0] = 16.0 * np.arange(N_CORES, dtype=np.float32)
    if run is not None:
        res = run({"volp": volp, "par": par})
        img8 = res["img"].reshape(N_CORES, W, HB)    # free index = w*16 + h
    else:
        from concourse.bass_utils import run_bass_kernel_spmd
        rows = 128 // N_CORES
        in_maps = [{"volp": volp[rows * k:rows * (k + 1)],
                    "par": par[k:k + 1]} for k in range(N_CORES)]
        res = run_bass_kernel_spmd(_CACHE["nc"], in_maps,
                                   list(range(N_CORES)))
        img8 = np.stack([res.results[k]["img"].reshape(W, HB)
                         for k in range(N_CORES)])
    img = img8.transpose(0, 2, 1).reshape(H, W)      # -> [H, W]
    return img[None].astype(np.float32)

